# revision 11
# baseline (speedup 1.0000x reference)
"""Trainium2 Bass kernel for nn_AlpamayoR1 (batched 65-point velocity-profile
least squares).

Math: per trajectory the reference solves (ATA + lam*D3'D3 + ridge I)[1:,1:] y =
rhs, a 64x64 SPD system.  ATA is tridiagonal with constant diagonal
[2,...,2,1] and off-diagonal c_j = cos(theta_j - theta_{j+1}); the smoothness
prior adds a tiny (1e-4-scale) bandwidth-3 tail.  We solve the exact
tridiagonal part T directly (twisted/two-ended, division-free determinant
recurrence + prefix scans for the substitutions) and absorb the band-2/3 tail
E with iterative refinement sweeps: resid_k = -E*delta_k exactly, so each
sweep is one elementwise band-apply plus one T-solve (two scans).

Distribution: pure data parallel, 1024 trajectories per NeuronCore (128 SBUF
partitions x 8 groups in the free dimension), 8 cores.
"""
import os
import numpy as np

import concourse.bacc as bacc
import concourse.mybir as mybir
import concourse.tile as tile
from concourse import bass_utils

F32 = mybir.dt.float32
OP = mybir.AluOpType
AF = mybir.ActivationFunctionType

P = 128          # SBUF partitions = trajectories per group
G = 8            # groups per core
NJ = 64          # unknowns per trajectory
SL = 34          # twisted segment length (positions 0..33)
GS = 70          # natural layout group stride
OFS = 3          # natural layout left pad (j=0 lives at col OFS)
N_CORES = 8
B_CORE = P * G   # 1024 trajectories per core
SWEEPS = int(os.environ.get("KERNEL_SWEEPS", "2"))

LAM, RIDGE, DT = 1e-4, 1e-4, 1.0
PI_2 = float(np.pi / 2)


def _host_consts():
    N = NJ
    D3 = np.zeros((N - 2, N + 1))
    r_ = np.arange(N - 2)
    D3[r_, r_] = -1.0
    D3[r_, r_ + 1] = 3.0
    D3[r_, r_ + 2] = -3.0
    D3[r_, r_ + 3] = 1.0
    DTD = (LAM / DT ** 6) * (D3.T @ D3)
    diagA = np.full(N + 1, 2.0)
    diagA[0] = 1.0
    diagA[-1] = 1.0
    a0 = diagA[1:] + np.diag(DTD)[1:] + RIDGE          # [64]
    e1 = np.diag(DTD, 1)[1:]                            # [63]
    dtd10 = DTD[1:4, 0].copy()                          # [3]
    return a0, e1, dtd10


A0, E1, DTD10 = _host_consts()
E1_INT = float(np.float32(E1[5]))        # interior off-diag const (-15e-4)


def f32c(x):
    return float(np.float32(x))


def _emit(nc, tc, pool, dxy_d, th_d, v0_d, out_d):
    _tagn = [0]

    def T(w):
        _tagn[0] += 1
        return pool.tile([P, w], F32, name=f"t{_tagn[0]}", tag=f"t{_tagn[0]}")

    TH = T(G * 65); COS = T(G * 65); SIN = T(G * 65)
    DXY = T(G * 128); SX = T(G * 128)
    V0 = T(G)
    CB = T(G * GS); OFF1 = T(G * GS); RHSN = T(G * GS)
    OFF1P = T(2 * G * SL); OSQ = T(2 * G * SL); RHSNP = T(2 * G * SL)
    NB = T(2 * G * SL); RN = T(2 * G * SL); RSCR = T(2 * G * SL)
    RB = T(2 * G * SL); LNEG = T(2 * G * SL)
    D0A = T(2 * G * SL); D0B = T(2 * G * SL); D1B = T(2 * G * SL)
    ZN = T(2 * G * SL); W = T(2 * G * SL); YB = T(2 * G * SL)
    GP = T(2 * G * SL); RSC = T(2 * G * SL)
    DBUF = T(G * GS); S2 = T(G * GS); S3 = T(G * GS); GG = T(G * GS)
    OUT = T(G * 65)
    JT = [T(G) for _ in range(6)]   # junction narrow scratch
    SCR = T(16)                      # factor step scratch [128, 2*8]

    # ---- 3./4.-D views ----
    def n3(buf, lo, hi, stride=GS, ofs=OFS):
        return buf[:, :].rearrange("p (g c) -> p g c", c=stride)[:, :, ofs + lo: ofs + hi]

    def n3r(buf, lo, hi, stride=GS, ofs=OFS):   # reversed j range hi-1..lo
        v = buf[:, :].rearrange("p (g c) -> p g c", c=stride)
        return v[:, :, ofs + hi - 1: (ofs + lo - 1) if ofs + lo >= 1 else None: -1]

    def p4(buf):
        return buf[:, :].rearrange("p (d g s) -> p d g s", d=2, s=SL)

    def ppos(buf, p_):    # [128, 2, 8] slice at position p, both dirs
        return p4(buf)[:, :, :, p_]

    def pd(buf, d, lo, hi, rev=False):
        v = p4(buf)
        if rev:
            return v[:, d, :, hi - 1: (lo - 1) if lo >= 1 else None: -1]
        return v[:, d, :, lo:hi]

    vv, gg_, ss, sy = nc.vector, nc.gpsimd, nc.scalar, nc.sync

    # ---- DMAs in ----
    th_v = th_d.ap().rearrange("(g p) j -> p g j", p=P)
    sy.dma_start(TH[:, :].rearrange("p (g j) -> p g j", j=65), th_v)
    dxy_v = dxy_d.ap().rearrange("(g p) j t -> p g (j t)", p=P)
    sy.dma_start(DXY[:, :].rearrange("p (g j) -> p g j", j=128), dxy_v)
    v0_v = v0_d.ap().rearrange("(g p) -> p g", p=P)
    sy.dma_start(V0[:, :], v0_v)

    # ---- init memsets (scan-traversed buffers must be finite/zero) ----
    for buf in (D0A, D0B, D1B, RHSNP, GP, OFF1P):
        vv.memset(buf[:, :], 0.0)
    gg_.memset(DBUF[:, :], 0.0)
    gg_.memset(S2[:, :], 0.0)
    gg_.memset(S3[:, :], 0.0)
    gg_.memset(GG[:, :], 0.0)

    # ---- trig (ACT); Sin LUT needs [-pi, pi]: wrap theta once (|theta|<3pi),
    # cos via sin(pi/2 - |theta_r|) which stays in [-pi/2, pi/2] ----
    PIT = pool.tile([P, 1], F32, name="pit", tag="pit")
    NEG1 = pool.tile([P, 1], F32, name="neg1", tag="neg1")
    THR = pool.tile([P, G * 65], F32, name="thr", tag="thr")
    MM = pool.tile([P, G * 65], F32, name="mm", tag="mm")
    vv.memset(PIT[:, :], PI_2)
    vv.memset(NEG1[:, :], -1.0)
    TWO_PI = float(2 * np.pi)
    vv.tensor_scalar(MM[:, :], TH[:, :], float(np.pi), None, OP.is_gt)
    vv.scalar_tensor_tensor(THR[:, :], MM[:, :], -TWO_PI, TH[:, :],
                            OP.mult, OP.add)
    vv.tensor_scalar(MM[:, :], TH[:, :], float(-np.pi), None, OP.is_lt)
    vv.scalar_tensor_tensor(THR[:, :], MM[:, :], TWO_PI, THR[:, :],
                            OP.mult, OP.add)
    ss.activation(SIN[:, :], THR[:, :], AF.Sin)
    vv.tensor_scalar(MM[:, :], THR[:, :], -1.0, None, OP.mult)
    vv.tensor_tensor(MM[:, :], MM[:, :], THR[:, :], OP.max)
    ss.activation(COS[:, :], MM[:, :], AF.Sin, bias=PIT[:, :], scale=NEG1[:, :])

    def c65(buf, lo, hi):
        return buf[:, :].rearrange("p (g j) -> p g j", j=65)[:, :, lo:hi]

    # ---- c_i = cos_i cos_{i+1} + sin_i sin_{i+1} (DVE, into CB j=0..63) ----
    vv.tensor_tensor(n3(CB, 0, 64), c65(COS, 0, 64), c65(COS, 1, 65), OP.mult)
    vv.tensor_tensor(n3(OFF1, 0, 64), c65(SIN, 0, 64), c65(SIN, 1, 65), OP.mult)
    vv.tensor_tensor(n3(CB, 0, 64), n3(CB, 0, 64), n3(OFF1, 0, 64), OP.add)

    # ---- OFF1_j = c_{j+1} + e1_j (interior const + 4 boundary fixes) ----
    vv.tensor_scalar(n3(OFF1, 0, 63), n3(CB, 1, 64), E1_INT, None, OP.add)
    for j in (0, 1, 61, 62):
        vv.tensor_scalar(n3(OFF1, j, j + 1).squeeze(), n3(OFF1, j, j + 1).squeeze(),
                         f32c(E1[j] - np.float32(E1_INT)), None, OP.add)

    # ---- rhsneg (POOL; needs COS/SIN + DXY) ----
    d3 = DXY[:, :].rearrange("p (g c) -> p g c", c=128)
    s3_ = SX[:, :].rearrange("p (g c) -> p g c", c=128)
    gg_.tensor_tensor(s3_[:, :, 0:126], d3[:, :, 0:126], d3[:, :, 2:128], OP.add)
    gg_.tensor_copy(s3_[:, :, 126:128], d3[:, :, 126:128])
    # m1 = cos_{j+1} * sx_j  -> RHSN ; m2 = sin_{j+1} * sy_j -> GG (scratch)
    gg_.tensor_tensor(n3(RHSN, 0, 64), c65(COS, 1, 65), s3_[:, :, 0:128:2], OP.mult)
    gg_.tensor_tensor(n3(GG, 0, 64), c65(SIN, 1, 65), s3_[:, :, 1:128:2], OP.mult)
    gg_.tensor_tensor(n3(RHSN, 0, 64), n3(RHSN, 0, 64), n3(GG, 0, 64), OP.add)
    gg_.tensor_scalar(n3(RHSN, 0, 64), n3(RHSN, 0, 64), -2.0, None, OP.mult)
    # v0 corrections (narrow): rhsneg_0 += c_0*v0 ; rhsneg_{0..2} += dtd10_j*v0
    gg_.tensor_tensor(JT[0][:, :], n3(CB, 0, 1).squeeze(), V0[:, :], OP.mult)
    gg_.tensor_tensor(n3(RHSN, 0, 1).squeeze(), n3(RHSN, 0, 1).squeeze(),
                      JT[0][:, :], OP.add)
    for j in range(3):
        gg_.tensor_scalar(JT[1][:, :], V0[:, :], f32c(DTD10[j]), None, OP.mult)
        gg_.tensor_tensor(n3(RHSN, j, j + 1).squeeze(),
                          n3(RHSN, j, j + 1).squeeze(), JT[1][:, :], OP.add)
    # pack rhs into twisted layout (fwd j=0..31; rev j=63..32)
    gg_.tensor_copy(pd(RHSNP, 0, 0, 32), n3(RHSN, 0, 32))
    gg_.tensor_copy(pd(RHSNP, 1, 0, 32), n3r(RHSN, 32, 64))

    # ---- OFF1 packed + OSQ (DVE) ----
    vv.tensor_copy(pd(OFF1P, 0, 0, 32), n3(OFF1, 0, 32))
    vv.tensor_copy(pd(OFF1P, 1, 0, 31), n3r(OFF1, 32, 63))
    vv.tensor_tensor(OSQ[:, :], OFF1P[:, :], OFF1P[:, :], OP.mult)

    # ---- factor: determinant recurrence N[p] = n_{p-1} ----
    nb4 = p4(NB)
    vv.memset(nb4[:, :, :, 0], 1.0)
    vv.memset(nb4[:, 0, :, 1], f32c(A0[0]))
    vv.memset(nb4[:, 1, :, 1], f32c(A0[63]))
    scr3 = SCR[:, :].rearrange("p (d g) -> p d g", d=2)
    for p_ in range(31):
        a0f, a0r = f32c(A0[p_ + 1]), f32c(A0[62 - p_])
        if a0f == a0r:
            vv.tensor_tensor(scr3[:, :, :], ppos(OSQ, p_), ppos(NB, p_), OP.mult)
            vv.scalar_tensor_tensor(ppos(NB, p_ + 2), ppos(NB, p_ + 1), a0f,
                                    scr3[:, :, :], OP.mult, OP.subtract)
        else:
            for d, imm in ((0, a0f), (1, a0r)):
                vv.tensor_tensor(scr3[:, d, :], p4(OSQ)[:, d, :, p_],
                                 p4(NB)[:, d, :, p_], OP.mult)
                vv.scalar_tensor_tensor(p4(NB)[:, d, :, p_ + 2],
                                        p4(NB)[:, d, :, p_ + 1], imm,
                                        scr3[:, d, :], OP.mult, OP.subtract)

    # ---- wide reciprocals: RN[p] = 1/N[p] (p=1..32); R[p] = N[p]*RN[p+1] ----
    def pseg(buf, lo, hi):   # [128, 16, hi-lo]: 2 free dims (segments, pos)
        return buf[:, :].rearrange("p (q s) -> p q s", s=SL)[:, :, lo:hi]

    vv.reciprocal_approx_accurate(pseg(RN, 1, 33), pseg(NB, 1, 33),
                                  pseg(RSCR, 1, 33))
    vv.tensor_tensor(p4(RB)[:, :, :, 0:32], p4(NB)[:, :, :, 0:32],
                     p4(RN)[:, :, :, 1:33], OP.mult)
    # LNEG = -OFF1P * R
    vv.scalar_tensor_tensor(p4(LNEG)[:, :, :, 0:32], p4(OFF1P)[:, :, :, 0:32],
                            -1.0, p4(RB)[:, :, :, 0:32], OP.mult, OP.mult)
    # D0A[p] = LNEG[p-1], p=1..32 (pos 0 and pads stay 0)
    vv.tensor_copy(p4(D0A)[:, :, :, 1:33], p4(LNEG)[:, :, :, 0:32])
    # D0B: fwd s=1..32 <- LNEG fwd 31..0 ; rev s=1..31 <- LNEG rev 30..0
    vv.tensor_copy(pd(D0B, 0, 1, 33), pd(LNEG, 0, 0, 32, rev=True))
    vv.tensor_copy(pd(D0B, 1, 1, 32), pd(LNEG, 1, 0, 31, rev=True))

    # ---- phase A scan: zneg ----
    vv.tensor_tensor_scan(ZN[:, :], D0A[:, :], RHSNP[:, :], 0.0, OP.mult, OP.add)

    # ---- junction (narrow [128,8] on DVE) ----
    o31f = p4(OSQ)[:, 0, :, 31]; r31f = p4(RB)[:, 0, :, 31]
    o30r = p4(OSQ)[:, 1, :, 30]; r30r = p4(RB)[:, 1, :, 30]
    vv.tensor_tensor(JT[0][:, :], o31f, r31f, OP.mult)
    vv.tensor_tensor(JT[1][:, :], o30r, r30r, OP.mult)
    vv.tensor_tensor(JT[0][:, :], JT[0][:, :], JT[1][:, :], OP.add)
    vv.tensor_scalar(JT[0][:, :], JT[0][:, :], -1.0, f32c(A0[32]),
                     OP.mult, OP.add)                      # d32
    vv.reciprocal(JT[1][:, :], JT[0][:, :])                # r32
    vv.tensor_scalar(JT[5][:, :], JT[1][:, :], -1e-4, None, OP.mult)  # r32sc
    l31f = p4(LNEG)[:, 0, :, 31]
    zn31f = p4(ZN)[:, 0, :, 31]; znp32 = p4(ZN)[:, 1, :, 31]
    vv.tensor_tensor(JT[2][:, :], l31f, zn31f, OP.mult)    # u
    vv.scalar_tensor_tensor(JT[3][:, :], znp32, -1.0, JT[2][:, :],
                            OP.mult, OP.subtract)          # z32
    vv.tensor_tensor(JT[4][:, :], JT[3][:, :], JT[1][:, :], OP.mult)  # y32

    # ---- W = -ZN*R ; build D1B; phase B scan -> YB ----
    vv.scalar_tensor_tensor(p4(W)[:, :, :, 0:32], p4(ZN)[:, :, :, 0:32], -1.0,
                            p4(RB)[:, :, :, 0:32], OP.mult, OP.mult)
    vv.tensor_copy(p4(D1B)[:, 0, :, 0], JT[4][:, :])
    vv.tensor_copy(p4(D1B)[:, 1, :, 0], JT[4][:, :])
    vv.tensor_copy(pd(D1B, 0, 1, 33), pd(W, 0, 0, 32, rev=True))
    vv.tensor_copy(pd(D1B, 1, 1, 32), pd(W, 1, 0, 31, rev=True))
    vv.tensor_tensor_scan(YB[:, :], D0B[:, :], D1B[:, :], 0.0, OP.mult, OP.add)

    # ---- unpack to DBUF (natural), OUT staging ----
    # fwd seg position s holds y_{32-s} (s=0..32); rev s holds y_{32+s} (s=0..31)
    vv.tensor_copy(n3r(DBUF, 0, 33), pd(YB, 0, 0, 33))
    vv.tensor_copy(n3(DBUF, 33, 64), pd(YB, 1, 1, 32))
    o3 = OUT[:, :].rearrange("p (g c) -> p g c", c=65)
    vv.tensor_copy(o3[:, :, 0], V0[:, :])
    vv.tensor_copy(o3[:, :, 1:65], n3(DBUF, 0, 64))

    # ---- refinement sweeps ----
    # RSC = -1e-4 * R (fold E's -1e-4 into w-scale)
    vv.tensor_scalar(p4(RSC)[:, :, :, 0:32], p4(RB)[:, :, :, 0:32], -1e-4,
                     None, OP.mult)
    for it in range(SWEEPS):
        # g = -6*(d<<2 + d>>2) + (d<<3 + d>>3)  in units of -1e-4
        gg_.tensor_tensor(n3(S2, 0, 64), n3(DBUF, 2, 66), n3(DBUF, -2, 62), OP.add)
        gg_.tensor_tensor(n3(S3, 0, 64), n3(DBUF, 3, 67), n3(DBUF, -3, 61), OP.add)
        gg_.tensor_scalar(n3(S2, 0, 64), n3(S2, 0, 64), -6.0, None, OP.mult)
        gg_.tensor_tensor(n3(GG, 0, 64), n3(S2, 0, 64), n3(S3, 0, 64), OP.add)
        # deviation fix: E[61,63]=E[63,61]=3e-4 (interior 6e-4): += 3*delta
        gg_.tensor_scalar(JT[1][:, :], n3(DBUF, 61, 62).squeeze(), 3.0, None, OP.mult)
        gg_.tensor_scalar(JT[2][:, :], n3(DBUF, 63, 64).squeeze(), 3.0, None, OP.mult)
        gg_.tensor_tensor(n3(GG, 63, 64).squeeze(), n3(GG, 63, 64).squeeze(),
                          JT[1][:, :], OP.add)
        gg_.tensor_tensor(n3(GG, 61, 62).squeeze(), n3(GG, 61, 62).squeeze(),
                          JT[2][:, :], OP.add)
        gg_.tensor_copy(pd(GP, 0, 0, 32), n3(GG, 0, 32))
        gg_.tensor_copy(pd(GP, 1, 0, 32), n3r(GG, 32, 64))
        # T-solve with data1 = GP, w-scale RSC, junction r32sc
        vv.tensor_tensor_scan(ZN[:, :], D0A[:, :], GP[:, :], 0.0, OP.mult, OP.add)
        zs31f = p4(ZN)[:, 0, :, 31]; zsp32 = p4(ZN)[:, 1, :, 31]
        vv.tensor_tensor(JT[2][:, :], l31f, zs31f, OP.mult)
        vv.scalar_tensor_tensor(JT[3][:, :], zsp32, -1.0, JT[2][:, :],
                                OP.mult, OP.subtract)
        vv.tensor_tensor(JT[4][:, :], JT[3][:, :], JT[5][:, :], OP.mult)  # y32s
        vv.scalar_tensor_tensor(p4(W)[:, :, :, 0:32], p4(ZN)[:, :, :, 0:32],
                                -1.0, p4(RSC)[:, :, :, 0:32], OP.mult, OP.mult)
        vv.tensor_copy(p4(D1B)[:, 0, :, 0], JT[4][:, :])
        vv.tensor_copy(p4(D1B)[:, 1, :, 0], JT[4][:, :])
        vv.tensor_copy(pd(D1B, 0, 1, 33), pd(W, 0, 0, 32, rev=True))
        vv.tensor_copy(pd(D1B, 1, 1, 32), pd(W, 1, 0, 31, rev=True))
        vv.tensor_tensor_scan(YB[:, :], D0B[:, :], D1B[:, :], 0.0, OP.mult, OP.add)
        vv.tensor_copy(n3r(DBUF, 0, 33), pd(YB, 0, 0, 33))
        vv.tensor_copy(n3(DBUF, 33, 64), pd(YB, 1, 1, 32))
        vv.tensor_tensor(o3[:, :, 1:65], o3[:, :, 1:65], n3(DBUF, 0, 64), OP.add)

    # ---- DMA out ----
    out_v = out_d.ap().rearrange("(g p) j -> p g j", p=P)
    sy.dma_start(out_v, OUT[:, :].rearrange("p (g j) -> p g j", j=65))


_PROG = None


def _build():
    global _PROG
    if _PROG is not None:
        return _PROG
    nc = bacc.Bacc("TRN2", target_bir_lowering=False, debug=False,
                   num_devices=N_CORES)
    dxy_d = nc.dram_tensor("dxy", [B_CORE, NJ, 2], F32, kind="ExternalInput")
    th_d = nc.dram_tensor("theta", [B_CORE, NJ + 1], F32, kind="ExternalInput")
    v0_d = nc.dram_tensor("v0", [B_CORE], F32, kind="ExternalInput")
    out_d = nc.dram_tensor("out", [B_CORE, NJ + 1], F32, kind="ExternalOutput")
    with tile.TileContext(nc) as tc:
        with tc.tile_pool(name="main", bufs=1) as pool:
            _emit(nc, tc, pool, dxy_d, th_d, v0_d, out_d)
    nc.compile()
    _PROG = nc
    return nc


def kernel(dxy, theta, v0):
    nc = _build()
    dxy = np.ascontiguousarray(np.asarray(dxy, dtype=np.float32))
    theta = np.ascontiguousarray(np.asarray(theta, dtype=np.float32))
    v0 = np.ascontiguousarray(np.asarray(v0, dtype=np.float32))
    B = dxy.shape[0]
    per = B // N_CORES
    assert per == B_CORE, (B, B_CORE)
    in_maps = [
        {"dxy": dxy[c * per:(c + 1) * per],
         "theta": theta[c * per:(c + 1) * per],
         "v0": v0[c * per:(c + 1) * per]}
        for c in range(N_CORES)
    ]
    res = bass_utils.run_bass_kernel_spmd(nc, in_maps, core_ids=list(range(N_CORES)))
    return np.concatenate([r["out"] for r in res.results], axis=0)


# revision 30
# speedup vs baseline: 1.5226x; 1.5226x over previous
"""Trainium2 Bass kernel for nn_AlpamayoR1 (batched 65-point velocity-profile
least squares).

Math: per trajectory the reference solves (ATA + lam*D3'D3 + ridge I)[1:,1:] y =
rhs, a 64x64 SPD system.  ATA is tridiagonal with constant diagonal
[2,...,2,1] and off-diagonal c_j = cos(theta_j - theta_{j+1}); the smoothness
prior adds a tiny (1e-4-scale) bandwidth-3 tail.  We solve the exact
tridiagonal part T directly (twisted/two-ended, division-free determinant
recurrence + prefix scans for the substitutions) and absorb the band-2/3 tail
E with iterative refinement sweeps: resid_k = -E*delta_k exactly, so each
sweep is one elementwise band-apply plus one T-solve (two scans).

Distribution: pure data parallel, 1024 trajectories per NeuronCore (128 SBUF
partitions x 8 groups in the free dimension), 8 cores.
"""
import os
import numpy as np

import concourse.bacc as bacc
import concourse.mybir as mybir
import concourse.tile as tile
from concourse import bass_utils

F32 = mybir.dt.float32
OP = mybir.AluOpType
AF = mybir.ActivationFunctionType

P = 128          # SBUF partitions = trajectories per group
G = 8            # groups per core
NJ = 64          # unknowns per trajectory
SL = 34          # twisted segment length (positions 0..33)
GS = 70          # natural layout group stride
OFS = 3          # natural layout left pad (j=0 lives at col OFS)
N_CORES = 8
B_CORE = P * G   # 1024 trajectories per core
SWEEPS = int(os.environ.get("KERNEL_SWEEPS", "1"))

LAM, RIDGE, DT = 1e-4, 1e-4, 1.0
PI_2 = float(np.pi / 2)


def _host_consts():
    N = NJ
    D3 = np.zeros((N - 2, N + 1))
    r_ = np.arange(N - 2)
    D3[r_, r_] = -1.0
    D3[r_, r_ + 1] = 3.0
    D3[r_, r_ + 2] = -3.0
    D3[r_, r_ + 3] = 1.0
    DTD = (LAM / DT ** 6) * (D3.T @ D3)
    diagA = np.full(N + 1, 2.0)
    diagA[0] = 1.0
    diagA[-1] = 1.0
    a0 = diagA[1:] + np.diag(DTD)[1:] + RIDGE          # [64]
    e1 = np.diag(DTD, 1)[1:]                            # [63]
    dtd10 = DTD[1:4, 0].copy()                          # [3]
    return a0, e1, dtd10


A0, E1, DTD10 = _host_consts()
E1_INT = float(np.float32(E1[5]))        # interior off-diag const (-15e-4)


def f32c(x):
    return float(np.float32(x))


def _emit(nc, tc, pool, dxy_d, th_d, v0_d, e1_d, out_d):
    _tagn = [0]

    def T(w):
        _tagn[0] += 1
        return pool.tile([P, w], F32, name=f"t{_tagn[0]}", tag=f"t{_tagn[0]}")

    TH = T(G * 65); COS = T(G * 65); SIN = T(G * 65)
    THR = T(G * 65); MM = T(G * 65)
    DXY = T(G * 128); SX = T(G * 128)
    V0 = T(G)
    CB = T(G * 64); RHSN = T(G * 64)
    E1PK = T(2 * G * SL)
    OFF1P = T(2 * G * SL); OSQ = T(2 * G * SL); RHSNP = T(2 * G * SL)
    NB = T(2 * G * SL); RN = T(2 * G * SL); RSCR = T(2 * G * SL)
    RB = T(2 * G * SL); LNEG = T(2 * G * SL)
    D0A = T(2 * G * SL); D0B = T(2 * G * SL); D1B = T(2 * G * SL)
    ZN = T(2 * G * SL); YB = T(2 * G * SL)
    GP = T(2 * G * SL); RSC = T(2 * G * SL)
    DBUF = T(G * GS + 8); DB2 = T(G * GS + 8)
    S2 = T(G * GS + 8); S3 = T(G * GS + 8); GG = T(G * GS + 8)
    JTT = T(8 * G)
    JT = [JTT[:, i * G:(i + 1) * G] for i in range(8)]
    SCR = T(16)
    PIT = pool.tile([P, 1], F32, name="pit", tag="pit")
    NEG1 = pool.tile([P, 1], F32, name="neg1", tag="neg1")
    MAGT = pool.tile([P, 1], F32, name="magt", tag="magt")
    INVT = pool.tile([P, 1], F32, name="invt", tag="invt")
    NMAGT = pool.tile([P, 1], F32, name="nmagt", tag="nmagt")

    # views ------------------------------------------------------------
    def nf(buf, lo, hi):          # flat natural view, col = g*GS + OFS + j
        return buf[:, lo:hi]

    def n3(buf, lo, hi):          # [128, 8, hi-lo] natural GS-strided
        return buf[:, :G * GS].rearrange("p (g c) -> p g c", c=GS)[:, :, OFS + lo: OFS + hi]

    def n3r(buf, lo, hi):
        v = buf[:, :G * GS].rearrange("p (g c) -> p g c", c=GS)
        return v[:, :, OFS + hi - 1: OFS + lo - 1 if OFS + lo >= 1 else None: -1]

    def c2(buf, lo, hi):          # [128, 8, hi-lo] of a 64-stride buffer
        return buf[:, :G * 64].rearrange("p (g c) -> p g c", c=64)[:, :, lo:hi]

    def c2r(buf, lo, hi):
        v = buf[:, :G * 64].rearrange("p (g c) -> p g c", c=64)
        return v[:, :, hi - 1: lo - 1 if lo >= 1 else None: -1]

    def c65(buf, lo, hi):
        return buf[:, :].rearrange("p (g j) -> p g j", j=65)[:, :, lo:hi]

    def p4(buf):
        return buf[:, :].rearrange("p (d g s) -> p d g s", d=2, s=SL)

    def ppos(buf, p_):
        return p4(buf)[:, :, :, p_]

    def pd(buf, d, lo, hi, rev=False):
        v = p4(buf)
        if rev:
            return v[:, d, :, hi - 1: lo - 1 if lo >= 1 else None: -1]
        return v[:, d, :, lo:hi]

    def pseg(buf, lo, hi):        # [128, 16, hi-lo]
        return buf[:, :].rearrange("p (q s) -> p q s", s=SL)[:, :, lo:hi]

    vv, gg_, ss, sy = nc.vector, nc.gpsimd, nc.scalar, nc.sync
    _sc = nc.named_scope

    # DMAs in -----------------------------------------------------------
    with _sc("dma_in"):
        th_v = th_d.ap().rearrange("(p g) j -> p g j", g=G)
        sy.dma_start(TH[:, :].rearrange("p (g j) -> p g j", j=65), th_v)
        dxy_v = dxy_d.ap().rearrange("(p g) j t -> p g (j t)", g=G)
        sy.dma_start(DXY[:, :].rearrange("p (g j) -> p g j", j=128), dxy_v)
        v0_v = v0_d.ap().rearrange("(p g) -> p g", g=G)
        sy.dma_start(V0[:, :], v0_v)
        sy.dma_start(E1PK[:, :], e1_d.ap())

    # init memsets (POOL, early; scan-traversed buffers must be finite)
    with _sc("memset"):
        for buf in (D0A, D0B, D1B, RHSNP, GP, OFF1P):
            gg_.memset(buf[:, :], 0.0)
        gg_.memset(DBUF[:, :], 0.0)
        gg_.memset(DB2[:, :], 0.0)
        vv.memset(PIT[:, :], PI_2)
        vv.memset(NEG1[:, :], -1.0)
        MAGIC = float(np.float32(1.5 * 2 ** 23))
        vv.memset(MAGT[:, :], MAGIC)
        vv.memset(INVT[:, :], float(1.0 / (2 * np.pi)))
        vv.memset(NMAGT[:, :], -MAGIC)

    # trig --------------------------------------------------------------
    # wrap via round-to-nearest using the fp32 magic-number trick
    # (valid since |theta/2pi| << 2^22): k = round(theta/2pi); thr = theta - 2pi*k
    with _sc("trig"):
        TWO_PI = float(2 * np.pi)
        MAGIC = float(np.float32(1.5 * 2 ** 23))
        # c_i = cos(theta_i - theta_{i+1}) directly: delta -> wrap -> |.| ->
        # Sin(pi/2 - |dr|).  This chain feeds pack/factor, so it goes first.
        SC = S2[:, 0:G * 64]
        SC2 = S3[:, 0:G * 64]
        vv.tensor_tensor(c2(CB, 0, 64), c65(TH, 0, 64), c65(TH, 1, 65),
                         OP.subtract)
        vv.tensor_scalar(SC, CB[:, :G * 64], float(1.0 / TWO_PI), MAGIC,
                         OP.mult, OP.add)
        vv.tensor_scalar(SC, SC, MAGIC, None, OP.subtract)
        vv.scalar_tensor_tensor(CB[:, :G * 64], SC, -TWO_PI, CB[:, :G * 64],
                                OP.mult, OP.add)
        vv.tensor_scalar(SC2, CB[:, :G * 64], -1.0, None, OP.mult)
        vv.tensor_tensor(CB[:, :G * 64], CB[:, :G * 64], SC2, OP.max)
        ss.activation(CB[:, :G * 64], CB[:, :G * 64], AF.Sin, bias=PIT[:, :],
                      scale=NEG1[:, :])
        # theta wrap for cos_t / sin_t (feeds the rhs path only)
        vv.tensor_scalar(MM[:, :], TH[:, :], float(1.0 / TWO_PI), MAGIC,
                         OP.mult, OP.add)
        vv.tensor_scalar(MM[:, :], MM[:, :], MAGIC, None, OP.subtract)
        vv.scalar_tensor_tensor(THR[:, :], MM[:, :], -TWO_PI, TH[:, :],
                                OP.mult, OP.add)
        ss.activation(MM[:, :], THR[:, :], AF.Abs)
        ss.activation(COS[:, :], MM[:, :], AF.Sin, bias=PIT[:, :], scale=NEG1[:, :])
        ss.activation(SIN[:, :], THR[:, :], AF.Sin)

    # OFF1P = c_{j+1} (packed twisted) + E1PK ; OSQ = OFF1P^2
    with _sc("pack"):
        vv.tensor_tensor(pd(OFF1P, 0, 0, 32), c2(CB, 1, 33),
                         pd(E1PK, 0, 0, 32), OP.add)
        vv.tensor_tensor(pd(OFF1P, 1, 0, 32, rev=True), c2(CB, 32, 64),
                         pd(E1PK, 1, 0, 32, rev=True), OP.add)
        vv.tensor_tensor(OSQ[:, :], OFF1P[:, :], OFF1P[:, :], OP.mult)

    # rhs (POOL, concurrent with factor): RHSN = cos_{j+1}*sx + sin_{j+1}*sy
    # (this is rhs/2; the factor-of-2 and sign fold into the w-scale)
    with _sc("rhs"):
        d3 = DXY[:, :].rearrange("p (g c) -> p g c", c=128)
        s3_ = SX[:, :].rearrange("p (g c) -> p g c", c=128)
        gg_.tensor_tensor(s3_[:, :, 0:126], d3[:, :, 0:126], d3[:, :, 2:128], OP.add)
        gg_.tensor_copy(s3_[:, :, 126:128], d3[:, :, 126:128])
        gg_.tensor_tensor(c2(RHSN, 0, 64), c65(COS, 1, 65),
                          s3_[:, :, 0:128:2], OP.mult)
        gg_.tensor_tensor(c2(GG, 0, 64), c65(SIN, 1, 65),
                          s3_[:, :, 1:128:2], OP.mult)
        gg_.tensor_tensor(RHSN[:, :], RHSN[:, :], GG[:, :G * 64], OP.add)
        # v0 fixes (in rhs/2 units, sign +: RHSN holds +rhs/2... corrections:
        # rhs_0 -= c_0 v0 -> RHSN_0 -= 0.5 c_0 v0 ; rhs_j -= dtd10_j v0 ->
        # RHSN_j -= 0.5 dtd10_j v0
        gg_.tensor_tensor(JT[0], c2(CB, 0, 1).squeeze(), V0[:, :], OP.mult)
        gg_.tensor_scalar(JT[0], JT[0], -0.5, None, OP.mult)
        gg_.tensor_tensor(c2(RHSN, 0, 1).squeeze(), c2(RHSN, 0, 1).squeeze(),
                          JT[0], OP.add)
        for j in range(3):
            gg_.tensor_scalar(JT[1], V0[:, :], f32c(-0.5 * DTD10[j]),
                              None, OP.mult)
            gg_.tensor_tensor(c2(RHSN, j, j + 1).squeeze(),
                              c2(RHSN, j, j + 1).squeeze(), JT[1], OP.add)
        gg_.tensor_copy(pd(RHSNP, 0, 0, 32), c2(RHSN, 0, 32))
        gg_.tensor_copy(pd(RHSNP, 1, 0, 32), c2r(RHSN, 32, 64))

    # factor: determinant recurrence ------------------------------------
    with _sc("factor"):
        nb4 = p4(NB)
        vv.memset(nb4[:, :, :, 0], 1.0)
        vv.memset(nb4[:, 0, :, 1], f32c(A0[0]))
        vv.memset(nb4[:, 1, :, 1], f32c(A0[63]))
        scrA = SCR[:, 0:16].rearrange("p (d g) -> p d g", d=2)
        scrB = RSCR[:, 0:16].rearrange("p (d g) -> p d g", d=2)
        for p_ in range(31):
            a0f, a0r = f32c(A0[p_ + 1]), f32c(A0[62 - p_])
            scr3 = scrA if (p_ % 2 == 0) else scrB
            if a0f == a0r:
                vv.tensor_tensor(scr3[:, :, :], ppos(OSQ, p_), ppos(NB, p_), OP.mult)
                vv.scalar_tensor_tensor(ppos(NB, p_ + 2), ppos(NB, p_ + 1), a0f,
                                        scr3[:, :, :], OP.mult, OP.subtract)
            else:
                for d, imm in ((0, a0f), (1, a0r)):
                    vv.tensor_tensor(scr3[:, d, :], p4(OSQ)[:, d, :, p_],
                                     p4(NB)[:, d, :, p_], OP.mult)
                    vv.scalar_tensor_tensor(p4(NB)[:, d, :, p_ + 2],
                                            p4(NB)[:, d, :, p_ + 1], imm,
                                            scr3[:, d, :], OP.mult, OP.subtract)

    # wide reciprocals, R, LNEG, scan coefficient buffers ----------------
    with _sc("postfac"):
        vv.reciprocal_approx_fast(pseg(RN, 1, 33), pseg(NB, 1, 33))
        vv.tensor_tensor(pseg(RB, 0, 32), pseg(NB, 0, 32), pseg(RN, 1, 33), OP.mult)
        vv.scalar_tensor_tensor(pseg(LNEG, 0, 32), pseg(OFF1P, 0, 32), -1.0,
                                pseg(RB, 0, 32), OP.mult, OP.mult)
        vv.tensor_copy(pseg(D0A, 1, 33), pseg(LNEG, 0, 32))
        vv.tensor_copy(pd(D0B, 0, 1, 33), pd(LNEG, 0, 0, 32, rev=True))
        vv.tensor_copy(pd(D0B, 1, 1, 32), pd(LNEG, 1, 0, 31, rev=True))

    # phase A scan ------------------------------------------------------
    with _sc("scanA"):
        vv.tensor_tensor_scan(ZN[:, :], D0A[:, :], RHSNP[:, :], 0.0,
                              OP.mult, OP.add)

    # junction ----------------------------------------------------------
    with _sc("junction"):
        o31f = p4(OSQ)[:, 0, :, 31]; r31f = p4(RB)[:, 0, :, 31]
        o30r = p4(OSQ)[:, 1, :, 30]; r30r = p4(RB)[:, 1, :, 30]
        vv.tensor_tensor(JT[0], o31f, r31f, OP.mult)
        vv.tensor_tensor(JT[1], o30r, r30r, OP.mult)
        vv.tensor_tensor(JT[0], JT[0], JT[1], OP.add)
        vv.tensor_scalar(JT[0], JT[0], -1.0, f32c(A0[32]),
                         OP.mult, OP.add)                      # d32
        vv.reciprocal(JT[1], JT[0])                # r32
        vv.tensor_scalar(JT[5], JT[1], -1e-4, None, OP.mult)  # r32sc
        vv.tensor_scalar(JT[6], JT[1], 2.0, None, OP.mult)    # r32*2
        l31f = p4(LNEG)[:, 0, :, 31]
        zn31f = p4(ZN)[:, 0, :, 31]; znp32 = p4(ZN)[:, 1, :, 31]
        vv.tensor_tensor(JT[2], l31f, zn31f, OP.mult)    # u = lneg31*zh31
        vv.tensor_tensor(JT[3], znp32, JT[2], OP.add)   # s
        vv.tensor_tensor(JT[4], JT[3], JT[6], OP.mult)  # y32
        vv.tensor_copy(p4(D1B)[:, 0, :, 0], JT[4])

    # w = 2*ZN*R written reversed straight into D1B; phase B scan -> YB --
    with _sc("scanB"):
        vv.scalar_tensor_tensor(pd(D1B, 0, 1, 33, rev=True), pd(ZN, 0, 0, 32),
                                2.0, pd(RB, 0, 0, 32), OP.mult, OP.mult)
        vv.scalar_tensor_tensor(pd(D1B, 1, 1, 32, rev=True), pd(ZN, 1, 0, 31),
                                2.0, pd(RB, 1, 0, 31), OP.mult, OP.mult)
        vv.tensor_copy(p4(D1B)[:, 1, :, 0], JT[4])
        vv.tensor_tensor_scan(YB[:, :], D0B[:, :], D1B[:, :], 0.0,
                              OP.mult, OP.add)

    # unpack to DBUF (natural padded layout) -----------------------------
    with _sc("unpack"):
        vv.tensor_copy(n3r(DBUF, 0, 33), pd(YB, 0, 0, 33))
        vv.tensor_copy(n3(DBUF, 33, 64), pd(YB, 1, 1, 32))

    # refinement sweeps --------------------------------------------------
    with _sc("sweeps"):
        if SWEEPS > 0:
            vv.tensor_scalar(pseg(RSC, 0, 32), pseg(RB, 0, 32), -1e-4,
                             None, OP.mult)
        for it in range(SWEEPS):
            SRC = DBUF if it == 0 else DB2
            NW = G * GS
            # flat shifted band apply: GG = -6*(d<<2 + d>>2) + (d<<3 + d>>3)
            vv.tensor_tensor(S2[:, 2:NW + 4], SRC[:, 4:NW + 6], SRC[:, 0:NW + 2],
                             OP.add)
            vv.tensor_tensor(S3[:, 3:NW + 3], SRC[:, 6:NW + 6], SRC[:, 0:NW],
                             OP.add)
            vv.scalar_tensor_tensor(pd(GP, 0, 0, 32), n3(S2, 0, 32), -6.0,
                                    n3(S3, 0, 32), OP.mult, OP.add)
            vv.scalar_tensor_tensor(pd(GP, 1, 0, 32), n3r(S2, 32, 64), -6.0,
                                    n3r(S3, 32, 64), OP.mult, OP.add)
            # E deviation fix at (61,63): read delta from YB (rev s=29,31)
            vv.scalar_tensor_tensor(p4(GP)[:, 1, :, 0], p4(YB)[:, 1, :, 29], 3.0,
                                    p4(GP)[:, 1, :, 0], OP.mult, OP.add)
            vv.scalar_tensor_tensor(p4(GP)[:, 1, :, 2], p4(YB)[:, 1, :, 31], 3.0,
                                    p4(GP)[:, 1, :, 2], OP.mult, OP.add)
            vv.tensor_tensor_scan(ZN[:, :], D0A[:, :], GP[:, :], 0.0,
                                  OP.mult, OP.add)
            zs31f = p4(ZN)[:, 0, :, 31]; zsp32 = p4(ZN)[:, 1, :, 31]
            vv.tensor_tensor(JT[2], l31f, zs31f, OP.mult)
            vv.scalar_tensor_tensor(JT[3], zsp32, -1.0, JT[2],
                                    OP.mult, OP.subtract)
            vv.tensor_tensor(JT[4], JT[3], JT[5], OP.mult)
            vv.tensor_copy(p4(D1B)[:, 0, :, 0], JT[4])
            vv.scalar_tensor_tensor(pd(D1B, 0, 1, 33, rev=True), pd(ZN, 0, 0, 32),
                                    -1.0, pd(RSC, 0, 0, 32), OP.mult, OP.mult)
            vv.scalar_tensor_tensor(pd(D1B, 1, 1, 32, rev=True), pd(ZN, 1, 0, 31),
                                    -1.0, pd(RSC, 1, 0, 31), OP.mult, OP.mult)
            vv.tensor_copy(p4(D1B)[:, 1, :, 0], JT[4])
            vv.tensor_tensor_scan(YB[:, :], D0B[:, :], D1B[:, :], 0.0,
                                  OP.mult, OP.add)
            if it + 1 < SWEEPS:
                vv.tensor_copy(n3r(DB2, 0, 33), pd(YB, 0, 0, 33))
                vv.tensor_copy(n3(DB2, 33, 64), pd(YB, 1, 1, 32))
                vv.tensor_tensor(DBUF[:, :NW], DBUF[:, :NW], DB2[:, :NW], OP.add)
            else:
                vv.tensor_tensor(n3r(DBUF, 0, 33), n3r(DBUF, 0, 33),
                                 pd(YB, 0, 0, 33), OP.add)
                vv.tensor_tensor(n3(DBUF, 33, 64), n3(DBUF, 33, 64),
                                 pd(YB, 1, 1, 32), OP.add)

    # DMA out ------------------------------------------------------------
    with _sc("dma_out"):
        outy = out_d.ap()[:, 1:65].rearrange("(p g) j -> p g j", g=G)
        sy.dma_start(outy, n3(DBUF, 0, 64))
        out0 = out_d.ap()[:, 0:1].rearrange("(p g) j -> p (g j)", g=G)
        sy.dma_start(out0, V0[:, :])




def _e1pk_host():
    """E1 (tridiag off-diag const) in packed twisted layout, broadcast to
    all partitions: fwd p=0..31 -> e1[p]; rev p=0..30 -> e1[62-p]."""
    row = np.zeros(2 * G * SL, np.float32)
    for g in range(G):
        for p_ in range(32):
            row[0 * G * SL + g * SL + p_] = E1[p_]
        for p_ in range(31):
            row[1 * G * SL + g * SL + p_] = E1[62 - p_]
    return np.ascontiguousarray(np.tile(row[None, :], (P, 1)))

_PROG = None


def _build():
    global _PROG
    if _PROG is not None:
        return _PROG
    nc = bacc.Bacc("TRN2", target_bir_lowering=False, debug=False,
                   num_devices=N_CORES)
    dxy_d = nc.dram_tensor("dxy", [B_CORE, NJ, 2], F32, kind="ExternalInput")
    th_d = nc.dram_tensor("theta", [B_CORE, NJ + 1], F32, kind="ExternalInput")
    v0_d = nc.dram_tensor("v0", [B_CORE], F32, kind="ExternalInput")
    e1_d = nc.dram_tensor("e1pk", [P, 2 * G * SL], F32, kind="ExternalInput")
    out_d = nc.dram_tensor("out", [B_CORE, NJ + 1], F32, kind="ExternalOutput")
    with tile.TileContext(nc) as tc:
        with tc.tile_pool(name="main", bufs=1) as pool:
            _emit(nc, tc, pool, dxy_d, th_d, v0_d, e1_d, out_d)
    nc.compile()
    _PROG = nc
    return nc


def kernel(dxy, theta, v0):
    nc = _build()
    dxy = np.ascontiguousarray(np.asarray(dxy, dtype=np.float32))
    theta = np.ascontiguousarray(np.asarray(theta, dtype=np.float32))
    v0 = np.ascontiguousarray(np.asarray(v0, dtype=np.float32))
    B = dxy.shape[0]
    per = B // N_CORES
    assert per == B_CORE, (B, B_CORE)
    e1pk = _e1pk_host()
    in_maps = [
        {"dxy": dxy[c * per:(c + 1) * per],
         "theta": theta[c * per:(c + 1) * per],
         "v0": v0[c * per:(c + 1) * per],
         "e1pk": e1pk}
        for c in range(N_CORES)
    ]
    res = bass_utils.run_bass_kernel_spmd(nc, in_maps, core_ids=list(range(N_CORES)))
    return np.concatenate([r["out"] for r in res.results], axis=0)
